# revision 31
# baseline (speedup 1.0000x reference)
"""Trainium2 Bass kernel for nn_GTCNN (product-graph GTCNN, 2 layers, K collapsed).

Math (per batch b, x: [M=8192, 32]):
  Adj = s0*I + s1*kron(I_t, As) + s2*kron(At, I_s) + s3*kron(At, As),  T=64, N=128
  h0 = x @ W1 + b1
  h_{l+1} = tanh((Adj @ h_l) @ Heff_l),   Heff_l = sum_k H[l, k]
  out = h2 @ W2 + b2

Device algorithm (Heff commutes with Adj, so Heff1 folds into W1 host-side):
  w  = x @ (W1 Heff1) + 1 (x) (b1 Heff1)          [FD matmul]
  z1 = tanh(P w + Q At-mix(w))                     [layer 1, all t]
  y  = P z1[:, q] + Q At[q,:]-mix(z1)              [layer 2, t-quarter]
  z2 = tanh(y @ Heff2);  out = z2 @ W2 + b2
  with P = s0*I + s1*As, Q = s2*I + s3*As folded on host.

Sharding: core c -> (b = c // 4, t-quarter q = c % 4). Layer 1 computed fully per
b (4x redundant; collectives have a ~10us floor, far above the redundant work).

Layouts (n = 32*nh + nl, t = 32*c + tl):
  NM  [n, t*32 + h]                      node-on-partition (P / Q matmuls)
  FD  [32*nh + h,  t*32 + nl]            feature-on-partition (W/Heff stationaries
                                         are block-diag kron(I4, W))
  FDT [32*nh + tl, ...]                  t-on-partition (At matmuls, stationaries
                                         kron(I4, At 32x32 block), PSUM-accum c)
All matmuls bf16 (PSUM fp32); PSUM evictions are scalar-engine copies casting to
bf16; layout moves are DVE 32x32 StreamTranspose ops on bf16 SBUF tiles. Every
transpose WRITES with stride-1 within-block (strided DVE writes cost ~3.6x);
consuming matmuls absorb the resulting layout via strided moving-AP views.
Tiles that land transposed-conventions: u_nm col = 64h+32cp+tl, g2 col =
1024c+32h+nl, u2_nm col = 32h+i.
"""

import numpy as np

T, NS, B, FIN, HID, FOUT = 64, 128, 2, 32, 32, 16
M = T * NS
NCORES, NQ = 8, 4
TQ = T // NQ  # 16 t's per quarter

_CACHE = {}

# bf16 weight pack [128, 1408]; 128-col slot i: atbd[2c+cp] i=0..3, atbq[c]
# i=4..5, P i=6, Q i=7, hi4_2 i=8, w2i4 i=9, w1hi4 i=10.
WPK_COLS = 1408


def _build_nc():
    from contextlib import ExitStack

    import concourse.mybir as mybir
    import concourse.tile as tile
    from concourse import bacc
    from concourse.bass import ds

    fp = mybir.dt.float32
    bf = mybir.dt.bfloat16
    AF = mybir.ActivationFunctionType

    nc = bacc.Bacc(
        "TRN2",
        target_bir_lowering=False,
        debug=False,
        enable_asserts=False,
        num_devices=NCORES,
    )

    xb = nc.dram_tensor("xb", [M, FIN], fp, kind="ExternalInput")
    w1h = nc.dram_tensor("w1h", [128, 128], fp, kind="ExternalInput")
    hi4f = nc.dram_tensor("hi4f", [128, 128], fp, kind="ExternalInput")
    bias2 = nc.dram_tensor("bias2", [128, 2], fp, kind="ExternalInput")
    wpk = nc.dram_tensor("wpk", [128, WPK_COLS], bf, kind="ExternalInput")
    outb = nc.dram_tensor("outb", [TQ * NS, FOUT], fp, kind="ExternalOutput")

    C512 = [slice(512 * j, 512 * (j + 1)) for j in range(4)]
    C1024 = [slice(1024 * j, 1024 * (j + 1)) for j in range(2)]

    with tile.TileContext(nc) as tc, ExitStack() as ctx:
        const = ctx.enter_context(tc.tile_pool(name="const", bufs=1))
        st = ctx.enter_context(tc.tile_pool(name="st", bufs=1))
        ps = ctx.enter_context(tc.tile_pool(name="ps", bufs=4, space="PSUM"))


        # ---- PE warm-up on an uninitialized tile: no input deps, so these
        # run at t~0 while DMAs stream, releasing the HAM clock-gate (PE is
        # ~2x slower until ~4us of sustained activity). Output never read.
        junk = const.tile([128, 512], bf, tag="junk")
        nc.gpsimd.memset(junk[:], 0)
        warm_ps = ps.tile([128, 512], fp, tag="big")
        for _ in range(20):
            nc.tensor.matmul(warm_ps[:], junk[:, 0:128], junk[:], start=True, stop=True)

        # ---- x load first (the critical path): NM [n, (t, f)], 4 t-chunks
        # split across the two HWDGE rings (sync + scalar queues).
        wpk_s = const.tile([128, WPK_COLS], bf, tag="wpk")
        nc.gpsimd.dma_start(wpk_s[:], wpk.ap())
        w1h_s = const.tile([128, 128], fp, tag="w1h")
        nc.gpsimd.dma_start(w1h_s[:], w1h.ap())
        hi4f_s = const.tile([128, 128], fp, tag="hi4f")
        nc.gpsimd.dma_start(hi4f_s[:], hi4f.ap())
        bias_s = const.tile([128, 2], fp, tag="bias")
        nc.gpsimd.dma_start(bias_s[:], bias2.ap())

        x_nm = st.tile([128, 2048], fp, tag="x_nm")
        xv = xb.ap().rearrange("(t n) f -> n t f", n=128)
        x_nm_v = x_nm[:].rearrange("p (t f) -> p t f", f=32)
        x_engs = [nc.sync, nc.scalar, nc.sync, nc.scalar, nc.sync, nc.scalar, nc.gpsimd, nc.gpsimd]
        for j in range(8):
            x_engs[j].dma_start(x_nm_v[:, 8 * j : 8 * (j + 1), :], xv[:, 8 * j : 8 * (j + 1), :])
        wslot = wpk_s[:].rearrange("p (i c) -> p i c", c=128)
        pmat = wslot[:, 6, :]
        qmat = wslot[:, 7, :]
        hi4_2 = wslot[:, 8, :]
        w2i4 = wslot[:, 9, :]

        # ---- per chunk: NM -> FD (DVE, fp32; casting on ACT would put the
        # x-DMA issue queue in front of the casts and stall the ladder) ----
        x_fd = st.tile([128, 2048], fp, tag="x_fd")
        for j in range(8):
            nc.vector.transpose(out=x_fd[:, 256 * j : 256 * (j + 1)], in_=x_nm[:, 256 * j : 256 * (j + 1)])

        def pe_keepalive(k):
            # Dep-free LDWEIGHTS on the junk tile: occupies the otherwise-idle
            # PE between matmul stages so the HAM clock-gate stays released.
            for _ in range(k):
                nc.tensor.ldweights(junk[:, 0:128])

        # ---- w = x @ W1H + b1H  (FD, fp32 matmuls), ACT bias-evict bf16.
        # The whole layer-1 midsection is interleaved at t-half (c) granularity
        # so the PE never idles long enough to re-engage the HAM throttle:
        # evict[c] -> {w_nm[c], g1[c]} -> u1 mms for contraction-half c ->
        # P mms for output-half c, with the u eviction/transpose and Q mms
        # trailing one half behind.
        wpre_h = [ps.tile([128, 1024], fp, tag="big", name=f"wpre{c}") for c in range(2)]
        for j in range(4):
            nc.tensor.matmul(
                wpre_h[j // 2][:, 512 * (j % 2) : 512 * (j % 2) + 512],
                w1h_s[:],
                x_fd[:, C512[j]],
                start=True,
                stop=True,
            )
        pe_keepalive(16)

        w_fd = st.tile([128, 2048], bf, tag="w_fd")
        w_nm = st.tile([128, 2048], bf, tag="w_nm")
        g1 = st.tile([128, 2048], bf, tag="g1")
        u_ps_h = [ps.tile([128, 1024], fp, tag="big", name=f"ups{c}") for c in range(2)]
        gi = w_fd[:].rearrange("p (c tl nl) -> p c nl tl", c=2, tl=32, nl=32)
        go = g1[:].rearrange("p (c nl h) -> p c nl h", c=2, nl=32, h=32)
        g1m = g1[:].rearrange("p (c nl h) -> p c nl h", c=2, nl=32, h=32)
        zpre_h = [None, None]

        for c in range(2):
            nc.scalar.activation(
                w_fd[:, C1024[c]], wpre_h[c][:], AF.Identity, bias=bias_s[:, 0:1]
            )
            nc.vector.transpose(out=go[:, c], in_=gi[:, c])
            nc.vector.transpose(out=w_nm[:, C1024[c]], in_=w_fd[:, C1024[c]])
            # u1 contributions from contraction-half c (both output halves).
            # Moving view streams (nl, h-half) so the innermost stride is 1
            # (strided innermost moving reads run the PE ~1.7x slower).
            for cp in range(2):
                for hh in range(2):
                    nc.tensor.matmul(
                        u_ps_h[cp][:, 512 * hh : 512 * (hh + 1)],
                        wslot[:, 2 * c + cp, :],
                        g1m[:, c, :, 16 * hh : 16 * (hh + 1)],
                        start=(c == 0),
                        stop=(c == 1),
                    )
            # P mms for output chunks of this half (needs only w_nm[c])
            zpre_h[c] = ps.tile([128, 1024], fp, tag="big", name=f"zpre{c}")
            for j in (2 * c, 2 * c + 1):
                nc.tensor.matmul(
                    zpre_h[c][:, 512 * (j % 2) : 512 * (j % 2) + 512],
                    pmat,
                    w_nm[:, C512[j]],
                    start=True,
                    stop=False,
                )

        # ---- ACT-evict u (cast bf16), FDT -> NM on DVE (u_nm t-inner:
        # col = 64h + 32cp + tl, so the transpose writes stride-1) ----
        u_fdt = st.tile([128, 2048], bf, tag="u_fdt")
        u_nm = st.tile([128, 2048], bf, tag="u_nm")
        # u_fdt col = 1024cp + 512hh + 16nl + hlow ; u_nm col = 1024hh + 64hlow
        # + 32cp + tl (t-inner for the stride-1 transpose write).
        ui = u_fdt[:].rearrange("p (cp hh nl h) -> p cp hh h nl", cp=2, hh=2, nl=32, h=16)
        uo = u_nm[:].rearrange("p (hh h cp tl) -> p cp hh h tl", hh=2, h=16, cp=2, tl=32)
        u_mv = u_nm[:].rearrange("p (hh h cp tl) -> p cp hh h tl", hh=2, h=16, cp=2, tl=32)
        for cp in range(2):
            nc.scalar.activation(u_fdt[:, C1024[cp]], u_ps_h[cp][:], AF.Identity)
            nc.vector.transpose(out=uo[:, cp], in_=ui[:, cp])
            # Q mms: stream (h, tl) from a stride-1 moving view into a strided
            # PSUM out AP that still lands zpre cols as (tl, h).
            for k in range(2):
                zq_out = zpre_h[cp][:, 512 * k : 512 * (k + 1)].rearrange(
                    "p (tl h) -> p h tl", tl=16, h=32
                )
                zq_o4 = zq_out.rearrange("p (hh h) tl -> p hh h tl", hh=2, h=16)
                nc.tensor.matmul(
                    zq_o4,
                    qmat,
                    u_mv[:, cp, :, :, 16 * k : 16 * (k + 1)],
                    start=False,
                    stop=True,
                )

        z1_nm = st.tile([128, 2048], bf, tag="z1_nm")
        for j in range(2):
            nc.scalar.activation(z1_nm[:, C1024[j]], zpre_h[j][:], AF.Tanh)

        # GPSIMD (idle) extracts this core's t-quarter of z1 so the layer-2
        # P-matmul gets a register-free moving AP (register APs on the PE cost
        # ~1.7us in TENSOR_LOADs right on the layer-2 critical path).
        pidg = nc.gpsimd.partition_id()
        toffg = (pidg % NQ) * TQ
        z1v = z1_nm[:].rearrange("p (t h) -> p t h", h=32)
        zq_cp = st.tile([128, 512], bf, tag="zq_cp")
        nc.gpsimd.tensor_copy(zq_cp[:], z1v[:, ds(toffg, TQ), :])

        # ====================== layer 2 (t-quarter only) ======================
        # g2 = FDT'(z1), stored nl-inner: col = 1024c + 32h + nl.
        g2 = st.tile([128, 2048], bf, tag="g2")
        zi = z1_nm[:].rearrange("p (c tl h) -> p c h tl", c=2, tl=32, h=32)
        zo = g2[:].rearrange("p (c h nl) -> p c h nl", c=2, h=32, nl=32)
        for c in range(2):
            nc.vector.transpose(out=zo[:, c], in_=zi[:, c])

        # zpre2 P-part first: runs on the PE while the u2 path's evict and
        # transpose are still in flight.
        zpre2 = ps.tile([128, 512], fp, tag="big")
        nc.tensor.matmul(zpre2[:], pmat, zq_cp[:], start=True, stop=False)

        # u2 = At[q rows]-mix(z1): out partitions (nh, tl' in 0..15)
        u2_ps = ps.tile([128, 1024], fp, tag="big")
        g2r = g2[:].rearrange("p (c h nl) -> p c h nl", c=2, h=32, nl=32)
        for c in range(2):
            for hh in range(2):
                nc.tensor.matmul(
                    u2_ps[:, 512 * hh : 512 * (hh + 1)],
                    wslot[:, 4 + c, :],
                    g2r[:, c, 16 * hh : 16 * (hh + 1), :],
                    start=(c == 0),
                    stop=(c == 1),
                )

        u2_f = st.tile([128, 1024], bf, tag="u2_f")
        nc.scalar.activation(u2_f[:], u2_ps[:], AF.Identity)

        # u2_nm stored i-inner (col = 32h + i), one 1024-el transpose.
        u2_nm = st.tile([128, 1024], bf, tag="u2_nm")
        u2i = u2_f[:].rearrange("p (h nl) -> p h nl", h=32, nl=32)
        u2o = u2_nm[:].rearrange("p (h i) -> p h i", h=32, i=32)
        nc.vector.transpose(out=u2o[:], in_=u2i[:])

        # zpre2 = P zq + Q u2 (NM quarter), evict bf16
        u2_mv = u2_nm[:].rearrange("p (h i) -> p h i", h=32, i=32)
        z2_out = zpre2[:].rearrange("p (tq h) -> p h tq", tq=16, h=32)
        nc.tensor.matmul(z2_out, qmat, u2_mv[:, :, 0:16], start=False, stop=True)

        # NM -> FD directly from PSUM (fp32 DVE transpose skips an ACT hop),
        # then the Heff2 matmul runs fp32.
        zq_fd = st.tile([128, 512], fp, tag="zq_fd")
        nc.vector.transpose(out=zq_fd[:], in_=zpre2[:])

        pre2 = ps.tile([128, 512], fp, tag="big")
        nc.tensor.matmul(pre2[:], hi4f_s[:], zq_fd[:], start=True, stop=True)
        h2_fd = st.tile([128, 512], bf, tag="h2_fd")
        nc.scalar.activation(h2_fd[:], pre2[:], AF.Tanh)

        opre = ps.tile([128, 512], fp, tag="big")
        nc.tensor.matmul(opre[:], w2i4, h2_fd[:], start=True, stop=True)
        out_fd = st.tile([128, 512], fp, tag="out_fd")
        nc.scalar.activation(out_fd[:], opre[:], AF.Identity, bias=bias_s[:, 1:2])

        out_nm = st.tile([128, 512], fp, tag="out_nm")
        onv = out_nm[:].rearrange("p (i j2) -> p i j2", j2=32)
        ov = outb.ap().rearrange("(i n) j -> n i j", n=128)
        for k in range(2):
            nc.vector.transpose(
                out=out_nm[:, 256 * k : 256 * (k + 1)], in_=out_fd[:, 256 * k : 256 * (k + 1)]
            )
            eng = nc.sync if k == 0 else nc.scalar
            eng.dma_start(ov[:, 8 * k : 8 * (k + 1), :], onv[:, 8 * k : 8 * (k + 1), 0:FOUT])

    nc.compile()
    return nc


def _host_weights(Adj_t, Adj_s, s, H, W1, b1, W2, b2):
    import ml_dtypes

    f4 = np.float32
    bf = ml_dtypes.bfloat16
    I4 = np.eye(4, dtype=f4)
    I128 = np.eye(128, dtype=f4)
    Heff = H.sum(axis=1).astype(f4)  # [2, 32, 32]

    P = (s[0] * I128 + s[1] * Adj_s).astype(f4)
    Q = (s[2] * I128 + s[3] * Adj_s).astype(f4)

    W1H = (W1 @ Heff[0]).astype(f4)
    b1H = (b1 @ Heff[0]).astype(f4)

    hi4_2 = np.kron(I4, Heff[1])
    w2pad = np.zeros((32, 32), dtype=f4)
    w2pad[:, :FOUT] = W2
    w2i4 = np.kron(I4, w2pad)

    bias2 = np.zeros((128, 2), dtype=f4)
    bias2[:, 0] = np.tile(b1H, 4)
    b2pad = np.zeros(32, dtype=f4)
    b2pad[:FOUT] = b2
    bias2[:, 1] = np.tile(b2pad, 4)

    wpk = np.zeros((NQ, 128, WPK_COLS), dtype=bf)
    for c in range(2):
        for cp in range(2):
            blk = np.kron(I4, Adj_t[32 * c : 32 * (c + 1), 32 * cp : 32 * (cp + 1)].astype(f4))
            wpk[:, :, 128 * (2 * c + cp) : 128 * (2 * c + cp + 1)] = blk.astype(bf)
    for q in range(NQ):
        for c in range(2):
            blk = np.zeros((32, 32), dtype=f4)
            blk[:, :TQ] = Adj_t[32 * c : 32 * (c + 1), TQ * q : TQ * (q + 1)]
            wpk[q, :, 128 * (4 + c) : 128 * (5 + c)] = np.kron(I4, blk).astype(bf)
    wpk[:, :, 128 * 6 : 128 * 7] = P.astype(bf)
    wpk[:, :, 128 * 7 : 128 * 8] = Q.astype(bf)
    wpk[:, :, 128 * 8 : 128 * 9] = hi4_2.astype(bf)
    wpk[:, :, 128 * 9 : 128 * 10] = w2i4.astype(bf)
    w1h = np.kron(I4, W1H)

    return w1h, hi4_2, bias2, wpk


def _in_maps(inputs):
    f4 = np.float32
    x = np.ascontiguousarray(np.asarray(inputs["x"], dtype=f4))
    w1h, hi4f, bias2, wpk = _host_weights(
        np.asarray(inputs["Adj_t"], dtype=f4),
        np.asarray(inputs["Adj_s"], dtype=f4),
        np.asarray(inputs["s"], dtype=f4),
        np.asarray(inputs["H"], dtype=f4),
        np.asarray(inputs["W1"], dtype=f4),
        np.asarray(inputs["b1"], dtype=f4),
        np.asarray(inputs["W2"], dtype=f4),
        np.asarray(inputs["b2"], dtype=f4),
    )
    maps = []
    for c in range(NCORES):
        b, q = c // NQ, c % NQ
        maps.append(
            {
                "xb": np.ascontiguousarray(x[b]),
                "w1h": w1h,
                "hi4f": np.ascontiguousarray(hi4f),
                "bias2": bias2,
                "wpk": np.ascontiguousarray(wpk[q]),
            }
        )
    return maps


def kernel(**inputs) -> np.ndarray:
    from concourse import bass_utils

    if "nc" not in _CACHE:
        _CACHE["nc"] = _build_nc()
    nc = _CACHE["nc"]

    maps = _in_maps(inputs)
    import os

    trace = bool(int(os.environ.get("GTCNN_TRACE", "0")))
    res = bass_utils.run_bass_kernel_spmd(
        nc,
        maps,
        core_ids=list(range(NCORES)),
        trace=trace,
        trace_cores=list(range(NCORES)) if trace else None,
        stitch_traces=False,
    )
    _CACHE["last_results"] = res

    out = np.empty((B, M, FOUT), dtype=np.float32)
    for c in range(NCORES):
        b, q = c // NQ, c % NQ
        out[b, 2048 * q : 2048 * (q + 1), :] = res.results[c]["outb"]
    return out


# revision 32
# speedup vs baseline: 1.0673x; 1.0673x over previous
"""Trainium2 Bass kernel for nn_GTCNN (product-graph GTCNN, 2 layers, K collapsed).

Math (per batch b, x: [M=8192, 32]):
  Adj = s0*I + s1*kron(I_t, As) + s2*kron(At, I_s) + s3*kron(At, As),  T=64, N=128
  h0 = x @ W1 + b1
  h_{l+1} = tanh((Adj @ h_l) @ Heff_l),   Heff_l = sum_k H[l, k]
  out = h2 @ W2 + b2

Device algorithm (Heff commutes with Adj, so Heff1 folds into W1 host-side):
  w  = x @ (W1 Heff1) + 1 (x) (b1 Heff1)          [FD matmul]
  z1 = tanh(P w + Q At-mix(w))                     [layer 1, all t]
  y  = P z1[:, q] + Q At[q,:]-mix(z1)              [layer 2, t-quarter]
  z2 = tanh(y @ Heff2);  out = z2 @ W2 + b2
  with P = s0*I + s1*As, Q = s2*I + s3*As folded on host.

Sharding: core c -> (b = c // 4, t-quarter q = c % 4). Layer 1 computed fully per
b (4x redundant; collectives have a ~10us floor, far above the redundant work).

Layouts (n = 32*nh + nl, t = 32*c + tl):
  NM  [n, t*32 + h]                      node-on-partition (P / Q matmuls)
  FD  [32*nh + h,  t*32 + nl]            feature-on-partition (W/Heff stationaries
                                         are block-diag kron(I4, W))
  FDT [32*nh + tl, ...]                  t-on-partition (At matmuls, stationaries
                                         kron(I4, At 32x32 block), PSUM-accum c)
All matmuls bf16 (PSUM fp32); PSUM evictions are scalar-engine copies casting to
bf16; layout moves are DVE 32x32 StreamTranspose ops on bf16 SBUF tiles. Every
transpose WRITES with stride-1 within-block (strided DVE writes cost ~3.6x);
consuming matmuls absorb the resulting layout via strided moving-AP views.
Tiles that land transposed-conventions: u_nm col = 64h+32cp+tl, g2 col =
1024c+32h+nl, u2_nm col = 32h+i.
"""

import numpy as np

T, NS, B, FIN, HID, FOUT = 64, 128, 2, 32, 32, 16
M = T * NS
NCORES, NQ = 8, 4
TQ = T // NQ  # 16 t's per quarter

_CACHE = {}

# bf16 weight pack [128, 1408]; 128-col slot i: atbd[2c+cp] i=0..3, atbq[c]
# i=4..5, P i=6, Q i=7, hi4_2 i=8, w2i4 i=9, w1hi4 i=10.
WPK_COLS = 1408


def _build_nc():
    from contextlib import ExitStack

    import concourse.mybir as mybir
    import concourse.tile as tile
    from concourse import bacc
    from concourse.bass import ds

    fp = mybir.dt.float32
    bf = mybir.dt.bfloat16
    AF = mybir.ActivationFunctionType

    nc = bacc.Bacc(
        "TRN2",
        target_bir_lowering=False,
        debug=False,
        enable_asserts=False,
        num_devices=NCORES,
    )

    xb = nc.dram_tensor("xb", [M, FIN], fp, kind="ExternalInput")
    w1h = nc.dram_tensor("w1h", [128, 128], fp, kind="ExternalInput")
    bias2 = nc.dram_tensor("bias2", [128, 2], fp, kind="ExternalInput")
    wpk = nc.dram_tensor("wpk", [128, WPK_COLS], bf, kind="ExternalInput")
    outb = nc.dram_tensor("outb", [TQ * NS, FOUT], fp, kind="ExternalOutput")

    C512 = [slice(512 * j, 512 * (j + 1)) for j in range(4)]
    C1024 = [slice(1024 * j, 1024 * (j + 1)) for j in range(2)]

    with tile.TileContext(nc) as tc, ExitStack() as ctx:
        const = ctx.enter_context(tc.tile_pool(name="const", bufs=1))
        st = ctx.enter_context(tc.tile_pool(name="st", bufs=1))
        ps = ctx.enter_context(tc.tile_pool(name="ps", bufs=4, space="PSUM"))


        # ---- PE warm-up on an uninitialized tile: no input deps, so these
        # run at t~0 while DMAs stream, releasing the HAM clock-gate (PE is
        # ~2x slower until ~4us of sustained activity). Output never read.
        junk = const.tile([128, 512], bf, tag="junk")
        nc.gpsimd.memset(junk[:], 0)
        warm_ps = ps.tile([128, 512], fp, tag="big")
        for _ in range(20):
            nc.tensor.matmul(warm_ps[:], junk[:, 0:128], junk[:], start=True, stop=True)

        # ---- x load first (the critical path): NM [n, (t, f)], 4 t-chunks
        # split across the two HWDGE rings (sync + scalar queues).
        wpk_s = const.tile([128, WPK_COLS], bf, tag="wpk")
        nc.gpsimd.dma_start(wpk_s[:], wpk.ap())
        w1h_s = const.tile([128, 128], fp, tag="w1h")
        nc.gpsimd.dma_start(w1h_s[:], w1h.ap())
        bias_s = const.tile([128, 2], fp, tag="bias")
        nc.gpsimd.dma_start(bias_s[:], bias2.ap())

        x_nm = st.tile([128, 2048], fp, tag="x_nm")
        xv = xb.ap().rearrange("(t n) f -> n t f", n=128)
        x_nm_v = x_nm[:].rearrange("p (t f) -> p t f", f=32)
        x_engs = [nc.sync, nc.scalar, nc.sync, nc.scalar, nc.sync, nc.scalar, nc.gpsimd, nc.gpsimd]
        for j in range(8):
            x_engs[j].dma_start(x_nm_v[:, 8 * j : 8 * (j + 1), :], xv[:, 8 * j : 8 * (j + 1), :])
        wslot = wpk_s[:].rearrange("p (i c) -> p i c", c=128)
        pmat = wslot[:, 6, :]
        qmat = wslot[:, 7, :]
        hi4_2 = wslot[:, 8, :]
        w2i4 = wslot[:, 9, :]

        # ---- per chunk: NM -> FD (DVE, fp32; casting on ACT would put the
        # x-DMA issue queue in front of the casts and stall the ladder) ----
        x_fd = st.tile([128, 2048], fp, tag="x_fd")
        for j in range(8):
            nc.vector.transpose(out=x_fd[:, 256 * j : 256 * (j + 1)], in_=x_nm[:, 256 * j : 256 * (j + 1)])

        def pe_keepalive(k):
            # Dep-free LDWEIGHTS on the junk tile: occupies the otherwise-idle
            # PE between matmul stages so the HAM clock-gate stays released.
            for _ in range(k):
                nc.tensor.ldweights(junk[:, 0:128])

        # ---- w = x @ W1H + b1H  (FD, fp32 matmuls), ACT bias-evict bf16.
        # The whole layer-1 midsection is interleaved at t-half (c) granularity
        # so the PE never idles long enough to re-engage the HAM throttle:
        # evict[c] -> {w_nm[c], g1[c]} -> u1 mms for contraction-half c ->
        # P mms for output-half c, with the u eviction/transpose and Q mms
        # trailing one half behind.
        wpre_h = [ps.tile([128, 1024], fp, tag="big", name=f"wpre{c}") for c in range(2)]
        for j in range(4):
            nc.tensor.matmul(
                wpre_h[j // 2][:, 512 * (j % 2) : 512 * (j % 2) + 512],
                w1h_s[:],
                x_fd[:, C512[j]],
                start=True,
                stop=True,
            )
        pe_keepalive(16)

        w_fd = st.tile([128, 2048], bf, tag="w_fd")
        w_nm = st.tile([128, 2048], bf, tag="w_nm")
        g1 = st.tile([128, 2048], bf, tag="g1")
        u_ps_h = [ps.tile([128, 1024], fp, tag="big", name=f"ups{c}") for c in range(2)]
        gi = w_fd[:].rearrange("p (c tl nl) -> p c nl tl", c=2, tl=32, nl=32)
        go = g1[:].rearrange("p (c nl h) -> p c nl h", c=2, nl=32, h=32)
        g1m = g1[:].rearrange("p (c nl h) -> p c nl h", c=2, nl=32, h=32)
        zpre_h = [None, None]

        for c in range(2):
            nc.scalar.activation(
                w_fd[:, C1024[c]], wpre_h[c][:], AF.Identity, bias=bias_s[:, 0:1]
            )
            nc.vector.transpose(out=go[:, c], in_=gi[:, c])
            nc.vector.transpose(out=w_nm[:, C1024[c]], in_=w_fd[:, C1024[c]])
            # u1 contributions from contraction-half c (both output halves).
            # Moving view streams (nl, h-half) so the innermost stride is 1
            # (strided innermost moving reads run the PE ~1.7x slower).
            for cp in range(2):
                for hh in range(2):
                    nc.tensor.matmul(
                        u_ps_h[cp][:, 512 * hh : 512 * (hh + 1)],
                        wslot[:, 2 * c + cp, :],
                        g1m[:, c, :, 16 * hh : 16 * (hh + 1)],
                        start=(c == 0),
                        stop=(c == 1),
                    )
            # P mms for output chunks of this half (needs only w_nm[c])
            zpre_h[c] = ps.tile([128, 1024], fp, tag="big", name=f"zpre{c}")
            for j in (2 * c, 2 * c + 1):
                nc.tensor.matmul(
                    zpre_h[c][:, 512 * (j % 2) : 512 * (j % 2) + 512],
                    pmat,
                    w_nm[:, C512[j]],
                    start=True,
                    stop=False,
                )

        # ---- ACT-evict u (cast bf16), FDT -> NM on DVE (u_nm t-inner:
        # col = 64h + 32cp + tl, so the transpose writes stride-1) ----
        u_fdt = st.tile([128, 2048], bf, tag="u_fdt")
        u_nm = st.tile([128, 2048], bf, tag="u_nm")
        # u_fdt col = 1024cp + 512hh + 16nl + hlow ; u_nm col = 1024hh + 64hlow
        # + 32cp + tl (t-inner for the stride-1 transpose write).
        ui = u_fdt[:].rearrange("p (cp hh nl h) -> p cp hh h nl", cp=2, hh=2, nl=32, h=16)
        uo = u_nm[:].rearrange("p (hh h cp tl) -> p cp hh h tl", hh=2, h=16, cp=2, tl=32)
        u_mv = u_nm[:].rearrange("p (hh h cp tl) -> p cp hh h tl", hh=2, h=16, cp=2, tl=32)
        for cp in range(2):
            nc.scalar.activation(u_fdt[:, C1024[cp]], u_ps_h[cp][:], AF.Identity)
            nc.vector.transpose(out=uo[:, cp], in_=ui[:, cp])
            # Q mms: stream (h, tl) from a stride-1 moving view into a strided
            # PSUM out AP that still lands zpre cols as (tl, h).
            for k in range(2):
                zq_out = zpre_h[cp][:, 512 * k : 512 * (k + 1)].rearrange(
                    "p (tl h) -> p h tl", tl=16, h=32
                )
                zq_o4 = zq_out.rearrange("p (hh h) tl -> p hh h tl", hh=2, h=16)
                nc.tensor.matmul(
                    zq_o4,
                    qmat,
                    u_mv[:, cp, :, :, 16 * k : 16 * (k + 1)],
                    start=False,
                    stop=True,
                )

        z1_nm = st.tile([128, 2048], bf, tag="z1_nm")
        for j in range(2):
            nc.scalar.activation(z1_nm[:, C1024[j]], zpre_h[j][:], AF.Tanh)

        # GPSIMD (idle) extracts this core's t-quarter of z1 so the layer-2
        # P-matmul gets a register-free moving AP (register APs on the PE cost
        # ~1.7us in TENSOR_LOADs right on the layer-2 critical path).
        pidg = nc.gpsimd.partition_id()
        toffg = (pidg % NQ) * TQ
        z1v = z1_nm[:].rearrange("p (t h) -> p t h", h=32)
        zq_cp = st.tile([128, 512], bf, tag="zq_cp")
        nc.gpsimd.tensor_copy(zq_cp[:], z1v[:, ds(toffg, TQ), :])

        # ====================== layer 2 (t-quarter only) ======================
        # g2 = FDT'(z1), stored nl-inner: col = 1024c + 32h + nl.
        g2 = st.tile([128, 2048], bf, tag="g2")
        zi = z1_nm[:].rearrange("p (c tl h) -> p c h tl", c=2, tl=32, h=32)
        zo = g2[:].rearrange("p (c h nl) -> p c h nl", c=2, h=32, nl=32)
        for c in range(2):
            nc.vector.transpose(out=zo[:, c], in_=zi[:, c])

        # zpre2 P-part first: runs on the PE while the u2 path's evict and
        # transpose are still in flight.
        zpre2 = ps.tile([128, 512], fp, tag="big")
        nc.tensor.matmul(zpre2[:], pmat, zq_cp[:], start=True, stop=False)

        # u2 = At[q rows]-mix(z1): out partitions (nh, tl' in 0..15)
        u2_ps = ps.tile([128, 1024], fp, tag="big")
        g2r = g2[:].rearrange("p (c h nl) -> p c h nl", c=2, h=32, nl=32)
        for hh in range(2):
            for c in range(2):
                nc.tensor.matmul(
                    u2_ps[:, 512 * hh : 512 * (hh + 1)],
                    wslot[:, 4 + c, :],
                    g2r[:, c, 16 * hh : 16 * (hh + 1), :],
                    start=(c == 0),
                    stop=(c == 1),
                )

        u2_f = st.tile([128, 1024], bf, tag="u2_f")
        nc.scalar.activation(u2_f[:], u2_ps[:], AF.Identity)

        # u2_nm stored i-inner (col = 32h + i), one 1024-el transpose.
        u2_nm = st.tile([128, 1024], bf, tag="u2_nm")
        u2i = u2_f[:].rearrange("p (h nl) -> p h nl", h=32, nl=32)
        u2o = u2_nm[:].rearrange("p (h i) -> p h i", h=32, i=32)
        nc.vector.transpose(out=u2o[:], in_=u2i[:])

        # zpre2 = P zq + Q u2 (NM quarter), evict bf16
        u2_mv = u2_nm[:].rearrange("p (h i) -> p h i", h=32, i=32)
        z2_out = zpre2[:].rearrange("p (tq h) -> p h tq", tq=16, h=32)
        nc.tensor.matmul(z2_out, qmat, u2_mv[:, :, 0:16], start=False, stop=True)

        zq_nm = st.tile([128, 512], bf, tag="zq_nm")
        nc.scalar.activation(zq_nm[:], zpre2[:], AF.Identity)

        # NM -> FD, Heff2 matmul + tanh, W2 matmul + bias, FD -> NM, DMA out
        zq_fd = st.tile([128, 512], bf, tag="zq_fd")
        nc.vector.transpose(out=zq_fd[:], in_=zq_nm[:])

        pre2 = ps.tile([128, 512], fp, tag="big")
        nc.tensor.matmul(pre2[:], hi4_2, zq_fd[:], start=True, stop=True)
        h2_fd = st.tile([128, 512], bf, tag="h2_fd")
        nc.scalar.activation(h2_fd[:], pre2[:], AF.Tanh)

        opre = ps.tile([128, 512], fp, tag="big")
        nc.tensor.matmul(opre[:], w2i4, h2_fd[:], start=True, stop=True)
        out_fd = st.tile([128, 512], fp, tag="out_fd")
        nc.scalar.activation(out_fd[:], opre[:], AF.Identity, bias=bias_s[:, 1:2])

        out_nm = st.tile([128, 512], fp, tag="out_nm")
        onv = out_nm[:].rearrange("p (i j2) -> p i j2", j2=32)
        ov = outb.ap().rearrange("(i n) j -> n i j", n=128)
        for k in range(2):
            nc.vector.transpose(
                out=out_nm[:, 256 * k : 256 * (k + 1)], in_=out_fd[:, 256 * k : 256 * (k + 1)]
            )
            eng = nc.sync if k == 0 else nc.scalar
            eng.dma_start(ov[:, 8 * k : 8 * (k + 1), :], onv[:, 8 * k : 8 * (k + 1), 0:FOUT])

    nc.compile()
    return nc


def _host_weights(Adj_t, Adj_s, s, H, W1, b1, W2, b2):
    import ml_dtypes

    f4 = np.float32
    bf = ml_dtypes.bfloat16
    I4 = np.eye(4, dtype=f4)
    I128 = np.eye(128, dtype=f4)
    Heff = H.sum(axis=1).astype(f4)  # [2, 32, 32]

    P = (s[0] * I128 + s[1] * Adj_s).astype(f4)
    Q = (s[2] * I128 + s[3] * Adj_s).astype(f4)

    W1H = (W1 @ Heff[0]).astype(f4)
    b1H = (b1 @ Heff[0]).astype(f4)

    hi4_2 = np.kron(I4, Heff[1])
    w2pad = np.zeros((32, 32), dtype=f4)
    w2pad[:, :FOUT] = W2
    w2i4 = np.kron(I4, w2pad)

    bias2 = np.zeros((128, 2), dtype=f4)
    bias2[:, 0] = np.tile(b1H, 4)
    b2pad = np.zeros(32, dtype=f4)
    b2pad[:FOUT] = b2
    bias2[:, 1] = np.tile(b2pad, 4)

    wpk = np.zeros((NQ, 128, WPK_COLS), dtype=bf)
    for c in range(2):
        for cp in range(2):
            blk = np.kron(I4, Adj_t[32 * c : 32 * (c + 1), 32 * cp : 32 * (cp + 1)].astype(f4))
            wpk[:, :, 128 * (2 * c + cp) : 128 * (2 * c + cp + 1)] = blk.astype(bf)
    for q in range(NQ):
        for c in range(2):
            blk = np.zeros((32, 32), dtype=f4)
            blk[:, :TQ] = Adj_t[32 * c : 32 * (c + 1), TQ * q : TQ * (q + 1)]
            wpk[q, :, 128 * (4 + c) : 128 * (5 + c)] = np.kron(I4, blk).astype(bf)
    wpk[:, :, 128 * 6 : 128 * 7] = P.astype(bf)
    wpk[:, :, 128 * 7 : 128 * 8] = Q.astype(bf)
    wpk[:, :, 128 * 8 : 128 * 9] = hi4_2.astype(bf)
    wpk[:, :, 128 * 9 : 128 * 10] = w2i4.astype(bf)
    w1h = np.kron(I4, W1H)

    return w1h, bias2, wpk


def _in_maps(inputs):
    f4 = np.float32
    x = np.ascontiguousarray(np.asarray(inputs["x"], dtype=f4))
    w1h, bias2, wpk = _host_weights(
        np.asarray(inputs["Adj_t"], dtype=f4),
        np.asarray(inputs["Adj_s"], dtype=f4),
        np.asarray(inputs["s"], dtype=f4),
        np.asarray(inputs["H"], dtype=f4),
        np.asarray(inputs["W1"], dtype=f4),
        np.asarray(inputs["b1"], dtype=f4),
        np.asarray(inputs["W2"], dtype=f4),
        np.asarray(inputs["b2"], dtype=f4),
    )
    maps = []
    for c in range(NCORES):
        b, q = c // NQ, c % NQ
        maps.append(
            {
                "xb": np.ascontiguousarray(x[b]),
                "w1h": w1h,
                "bias2": bias2,
                "wpk": np.ascontiguousarray(wpk[q]),
            }
        )
    return maps


def kernel(**inputs) -> np.ndarray:
    from concourse import bass_utils

    if "nc" not in _CACHE:
        _CACHE["nc"] = _build_nc()
    nc = _CACHE["nc"]

    maps = _in_maps(inputs)
    import os

    trace = bool(int(os.environ.get("GTCNN_TRACE", "0")))
    res = bass_utils.run_bass_kernel_spmd(
        nc,
        maps,
        core_ids=list(range(NCORES)),
        trace=trace,
        trace_cores=list(range(NCORES)) if trace else None,
        stitch_traces=False,
    )
    _CACHE["last_results"] = res

    out = np.empty((B, M, FOUT), dtype=np.float32)
    for c in range(NCORES):
        b, q = c // NQ, c % NQ
        out[b, 2048 * q : 2048 * (q + 1), :] = res.results[c]["outb"]
    return out
